# revision 5
# baseline (speedup 1.0000x reference)
"""Trainium2 Bass kernel for nn_CircuitLayer (GNN message passing / KCL circuit).

res[b, n] = sum over edges e: (+i_e at des, -i_e at src),
i_e = a_e * tanh(w_e * (v_src - v_des) + b_e),  v = [0, x][node]

Strategy (node-parallel over 8 NeuronCores):
  - Node slots [0, 50176) split: NC i owns 6272 slots (8 Q7 cores x 784 nodes,
    28 tiles of 28 nodes each).
  - Every edge-endpoint incidence is routed to the (NC, core, tile) owning its
    node, sorted/grouped by node; sign folding: src-incidence w'=+w, a'=-a;
    des-incidence w'=-w, a'=+a; contribution c = a'*tanh(w'*(v_own-v_other)+b).
  - Per tile the device: ap_gathers v_other/v_own from a per-core compact table
    (distinct endpoints, int16-indexable), computes c on DVE/ACT (bf16),
    prefix-scans c (f32 state) and gathers per-node segment boundary sums.
  - Per-NC outputs are disjoint node ranges -> no collective needed.
"""

import numpy as np

B, N, E = 16, 50000, 1600000
NN = N + 1
NCS = 8
QC = 8
NPT = 28
TPC = 28
ROUNDS = 4
TPR = TPC // ROUNDS
NPC = NPT * TPC          # 784 nodes per core
NPNC = NPC * QC          # 6272 node slots per NC
MAX_CLEN = 32768 - 16

_cache = {}


def _pad16(n):
    return (n + 15) & ~15


def _bf16(x):
    x = np.ascontiguousarray(x, np.float32)
    u = x.view(np.uint32)
    r = ((u >> 16) & 1) + 0x7FFF
    return ((u + r) & 0xFFFF0000).view(np.float32)


def _wrap16(v):
    # (S,) -> (16, S//16): out[p, s] = v[s*16 + p]
    return v.reshape(-1, 16).T.copy()


def _preprocess(x, param, src_node, des_node):
    import ml_dtypes

    src = np.asarray(src_node).astype(np.int64)
    des = np.asarray(des_node).astype(np.int64)
    a, w, b = (np.asarray(param[i], np.float32) for i in range(3))

    own = np.concatenate([src, des])
    other = np.concatenate([des, src])
    wp = np.concatenate([w, -w])
    ap_ = np.concatenate([-a, a])
    bp = np.concatenate([b, b])

    order = np.argsort(own, kind="stable")
    own, other = own[order], other[order]
    wp, ap_, bp = wp[order], ap_[order], bp[order]

    cnt = np.bincount(own, minlength=NN).astype(np.int64)
    cstart = np.zeros(NN + 1, np.int64)
    np.cumsum(cnt, out=cstart[1:])

    # global tile capacity
    tile_tot = np.bincount(np.arange(NN) // NPT, weights=cnt,
                           minlength=(NCS * QC * TPC))
    CAP = _pad16(int(tile_tot.max()) + 1 + 16)
    assert CAP <= 4096, CAP

    aux = np.concatenate([np.zeros((B, 1), np.float32),
                          np.asarray(x, np.float32)], axis=1)

    # ---- per (nc, core, round): distinct endpoint lists ----
    dls = [[[None] * QC for _ in range(ROUNDS)] for _ in range(NCS)]
    clen_need = 0
    for nc in range(NCS):
        for r in range(ROUNDS):
            for k in range(QC):
                n0 = nc * NPNC + k * NPC + r * TPR * NPT
                n1 = min(n0 + TPR * NPT, NN)
                if n0 >= NN:
                    dls[nc][r][k] = np.empty(0, np.int64)
                    continue
                s, e = cstart[n0], cstart[n1]
                u = np.unique(np.concatenate([other[s:e], own[s:e]]))
                dls[nc][r][k] = u
                clen_need = max(clen_need, len(u))
    CLEN = _pad16(clen_need)
    assert CLEN <= MAX_CLEN, CLEN

    IDXW = CAP // 16
    per_nc = []
    for nc in range(NCS):
        ctab = np.zeros((ROUNDS, 128, CLEN), np.float32)
        idxs = np.zeros((TPC, 128, 2 * IDXW + 2), np.int16)
        prm = np.zeros((TPC, 128, 3 * CAP), np.float32)
        for r in range(ROUNDS):
            for k in range(QC):
                dl = dls[nc][r][k]
                if len(dl):
                    ctab[r, 16 * k:16 * k + 16, :len(dl)] = aux[:, dl]
                for ti in range(TPR):
                    t = r * TPR + ti
                    n0 = nc * NPNC + k * NPC + t * NPT
                    ob = np.zeros(CAP, np.int16)
                    nb = np.zeros(CAP, np.int16)
                    wrow = np.zeros(CAP, np.float32)
                    brow = np.zeros(CAP, np.float32)
                    arow = np.zeros(CAP, np.float32)
                    cnts = np.zeros(NPT, np.int64)
                    if n0 < NN:
                        n1 = min(n0 + NPT, NN)
                        s, e = cstart[n0], cstart[n1]
                        m = e - s
                        assert m + 1 <= CAP
                        ob[1:1 + m] = np.searchsorted(dl, other[s:e])
                        nb[1:1 + m] = np.searchsorted(dl, own[s:e])
                        wrow[1:1 + m] = wp[s:e]
                        brow[1:1 + m] = bp[s:e]
                        arow[1:1 + m] = ap_[s:e]
                        cnts[:n1 - n0] = cnt[n0:n1]
                    ends = np.zeros(32, np.int16)
                    ends[:NPT] = np.cumsum(cnts).astype(np.int16)
                    sl = slice(16 * k, 16 * k + 16)
                    idxs[t, sl, 0:IDXW] = _wrap16(ob)
                    idxs[t, sl, IDXW:2 * IDXW] = _wrap16(nb)
                    idxs[t, sl, 2 * IDXW:] = _wrap16(ends)
                    prm[t, sl, 0:CAP] = wrow
                    prm[t, sl, CAP:2 * CAP] = brow
                    prm[t, sl, 2 * CAP:] = arow
        per_nc.append(dict(
            ctab=ctab,
            idxs=idxs,
            prm=_bf16(prm).astype(ml_dtypes.bfloat16),
        ))
    return dict(CAP=CAP, CLEN=CLEN), per_nc


def _build_program(CAP, CLEN):
    import sys
    if "/opt/trn_rl_repo" not in sys.path:
        sys.path.insert(0, "/opt/trn_rl_repo")
    from contextlib import ExitStack
    from concourse import bass, bacc, mybir, tile

    f32 = mybir.dt.float32
    bf16 = mybir.dt.bfloat16
    i16 = mybir.dt.int16
    Alu = mybir.AluOpType
    IDXW = CAP // 16

    nc = bacc.Bacc("TRN2", target_bir_lowering=False, debug=False,
                   num_devices=NCS)
    ctab_d = nc.dram_tensor("ctab_in", [ROUNDS, 128, CLEN], f32,
                            kind="ExternalInput")
    idxs_d = nc.dram_tensor("idxs_in", [TPC, 128, 2 * IDXW + 2], i16,
                            kind="ExternalInput")
    prm_d = nc.dram_tensor("prm_in", [TPC, 128, 3 * CAP], bf16,
                           kind="ExternalInput")
    out_d = nc.dram_tensor("res_out", [128, TPC * NPT], f32,
                           kind="ExternalOutput")

    with tile.TileContext(nc) as tc, ExitStack() as ctx:
        ctab_p = ctx.enter_context(tc.tile_pool(name="ctab", bufs=1))
        gat_p = ctx.enter_context(tc.tile_pool(name="gat", bufs=2))
        in_p = ctx.enter_context(tc.tile_pool(name="inp", bufs=2))
        zz_p = ctx.enter_context(tc.tile_pool(name="zz", bufs=2))
        p_p = ctx.enter_context(tc.tile_pool(name="pp", bufs=2))
        e_p = ctx.enter_context(tc.tile_pool(name="ee", bufs=2))
        res_p = ctx.enter_context(tc.tile_pool(name="res", bufs=1))

        res = res_p.tile([128, TPC * NPT], f32, tag="res")
        for r in range(ROUNDS):
            ctab = ctab_p.tile([128, CLEN], f32, tag="ctab")
            nc.sync.dma_start(ctab[:], ctab_d.ap()[r])
            for ti in range(TPR):
                t = r * TPR + ti
                idx = in_p.tile([128, 2 * IDXW + 2], i16, tag="idx")
                nc.sync.dma_start(idx[:], idxs_d.ap()[t])
                prm = in_p.tile([128, 3 * CAP], bf16, tag="prm")
                nc.sync.dma_start(prm[:], prm_d.ap()[t])

                go = gat_p.tile([128, CAP], f32, tag="go")
                gn = gat_p.tile([128, CAP], f32, tag="gn")
                nc.gpsimd.ap_gather(go[:], ctab[:], idx[:, 0:IDXW],
                                    128, CLEN, 1, CAP)
                nc.gpsimd.ap_gather(gn[:], ctab[:], idx[:, IDXW:2 * IDXW],
                                    128, CLEN, 1, CAP)

                z1 = zz_p.tile([128, CAP], bf16, tag="zz")
                nc.vector.tensor_tensor(z1[:], gn[:], go[:], Alu.subtract)
                z2 = zz_p.tile([128, CAP], bf16, tag="zz")
                nc.vector.tensor_tensor(z2[:], z1[:], prm[:, 0:CAP], Alu.mult)
                z3 = zz_p.tile([128, CAP], bf16, tag="zz")
                nc.vector.tensor_tensor(z3[:], z2[:], prm[:, CAP:2 * CAP],
                                        Alu.add)
                th = zz_p.tile([128, CAP], bf16, tag="zz")
                nc.scalar.activation(th[:], z3[:],
                                     mybir.ActivationFunctionType.Tanh)
                cc = zz_p.tile([128, CAP], bf16, tag="zz")
                nc.vector.tensor_tensor(cc[:], th[:], prm[:, 2 * CAP:],
                                        Alu.mult)
                P = p_p.tile([128, CAP], f32, tag="P")
                nc.vector.tensor_tensor_scan(P[:], cc[:], cc[:], 0.0,
                                             Alu.add, Alu.bypass)
                Eb = e_p.tile([128, 48], f32, tag="Eb")
                nc.vector.memset(Eb[:, 0:1], 0.0)
                nc.gpsimd.ap_gather(Eb[:, 1:33], P[:],
                                    idx[:, 2 * IDXW:2 * IDXW + 2],
                                    128, CAP, 1, 32)
                nc.vector.tensor_tensor(res[:, t * NPT:(t + 1) * NPT],
                                        Eb[:, 1:1 + NPT], Eb[:, 0:NPT],
                                        Alu.subtract)
        nc.sync.dma_start(out_d.ap()[:], res[:])
    nc.compile()
    return nc


def kernel(**inputs) -> np.ndarray:
    import sys
    if "/opt/trn_rl_repo" not in sys.path:
        sys.path.insert(0, "/opt/trn_rl_repo")
    from concourse.bass_utils import run_bass_kernel_spmd

    x = np.asarray(inputs["x"], np.float32)
    param = np.asarray(inputs["param"], np.float32)
    meta, per_nc = _preprocess(x, param, inputs["src_node"],
                               inputs["des_node"])
    key = (meta["CAP"], meta["CLEN"])
    if key not in _cache:
        _cache[key] = _build_program(*key)
    nc = _cache[key]

    in_maps = [{"ctab_in": d["ctab"], "idxs_in": d["idxs"],
                "prm_in": d["prm"]} for d in per_nc]
    results = run_bass_kernel_spmd(nc, in_maps, list(range(NCS))).results

    full = np.zeros((B, NCS * NPNC), np.float32)
    for i, om in enumerate(results):
        o = om["res_out"]
        for k in range(QC):
            full[:, i * NPNC + k * NPC:i * NPNC + (k + 1) * NPC] = \
                o[16 * k:16 * k + 16]
    return np.ascontiguousarray(full[:, 1:NN])


# revision 6
# speedup vs baseline: 33.9060x; 33.9060x over previous
"""Trainium2 Bass kernel for nn_CircuitLayer (GNN message passing / KCL circuit).

res[b, n] = sum over edges e: (+i_e at des, -i_e at src),
i_e = a_e * tanh(w_e * (v_src - v_des) + b_e),  v = [0, x][node]

Strategy (node-parallel over 8 NeuronCores):
  - Node slots [0, 50176) split: NC i owns 6272 slots (8 Q7 cores x 784 nodes,
    28 tiles of 28 nodes each).
  - Every edge-endpoint incidence is routed to the (NC, core, tile) owning its
    node, sorted/grouped by node; sign folding: src-incidence w'=+w, a'=-a;
    des-incidence w'=-w, a'=+a; contribution c = a'*tanh(w'*(v_own-v_other)+b).
  - Per tile the device: ap_gathers v_other/v_own from a per-core compact table
    (distinct endpoints, int16-indexable), computes c on DVE/ACT (bf16),
    prefix-scans c (f32 state) and gathers per-node segment boundary sums.
  - Per-NC outputs are disjoint node ranges -> no collective needed.
"""

import numpy as np

B, N, E = 16, 50000, 1600000
NN = N + 1
NCS = 8
QC = 8
NPT = 28
TPC = 28
ROUNDS = 4
TPR = TPC // ROUNDS
NPC = NPT * TPC          # 784 nodes per core
NPNC = NPC * QC          # 6272 node slots per NC
MAX_CLEN = 32768 - 16

_cache = {}


def _pad16(n):
    return (n + 15) & ~15


def _bf16(x):
    x = np.ascontiguousarray(x, np.float32)
    u = x.view(np.uint32)
    r = ((u >> 16) & 1) + 0x7FFF
    return ((u + r) & 0xFFFF0000).view(np.float32)


def _wrap16(v):
    # (S,) -> (16, S//16): out[p, s] = v[s*16 + p]
    return v.reshape(-1, 16).T.copy()


def _preprocess(x, param, src_node, des_node):
    import ml_dtypes

    src = np.asarray(src_node).astype(np.int64)
    des = np.asarray(des_node).astype(np.int64)
    a, w, b = (np.asarray(param[i], np.float32) for i in range(3))

    own = np.concatenate([src, des])
    other = np.concatenate([des, src])
    wp = np.concatenate([w, -w])
    ap_ = np.concatenate([-a, a])
    bp = np.concatenate([b, b])

    order = np.argsort(own, kind="stable")
    own, other = own[order], other[order]
    wp, ap_, bp = wp[order], ap_[order], bp[order]

    cnt = np.bincount(own, minlength=NN).astype(np.int64)
    cstart = np.zeros(NN + 1, np.int64)
    np.cumsum(cnt, out=cstart[1:])

    # global tile capacity
    tile_tot = np.bincount(np.arange(NN) // NPT, weights=cnt,
                           minlength=(NCS * QC * TPC))
    CAP = _pad16(int(tile_tot.max()) + 1 + 16)
    assert CAP <= 4096, CAP

    aux = np.concatenate([np.zeros((B, 1), np.float32),
                          np.asarray(x, np.float32)], axis=1)

    # ---- per (nc, core, round): distinct endpoint lists ----
    dls = [[[None] * QC for _ in range(ROUNDS)] for _ in range(NCS)]
    clen_need = 0
    for nc in range(NCS):
        for r in range(ROUNDS):
            for k in range(QC):
                n0 = nc * NPNC + k * NPC + r * TPR * NPT
                n1 = min(n0 + TPR * NPT, NN)
                if n0 >= NN:
                    dls[nc][r][k] = np.empty(0, np.int64)
                    continue
                s, e = cstart[n0], cstart[n1]
                u = np.unique(np.concatenate([other[s:e], own[s:e]]))
                dls[nc][r][k] = u
                clen_need = max(clen_need, len(u))
    CLEN = _pad16(clen_need)
    assert CLEN <= MAX_CLEN, CLEN

    IDXW = CAP // 16
    per_nc = []
    for nc in range(NCS):
        ctab = np.zeros((ROUNDS, 128, CLEN), np.float32)
        idxs = np.zeros((TPC, 128, 2 * IDXW + 2), np.int16)
        prm = np.zeros((TPC, 128, 3 * CAP), np.float32)
        for r in range(ROUNDS):
            for k in range(QC):
                dl = dls[nc][r][k]
                if len(dl):
                    ctab[r, 16 * k:16 * k + 16, :len(dl)] = aux[:, dl]
                for ti in range(TPR):
                    t = r * TPR + ti
                    n0 = nc * NPNC + k * NPC + t * NPT
                    ob = np.zeros(CAP, np.int16)
                    nb = np.zeros(CAP, np.int16)
                    wrow = np.zeros(CAP, np.float32)
                    brow = np.zeros(CAP, np.float32)
                    arow = np.zeros(CAP, np.float32)
                    cnts = np.zeros(NPT, np.int64)
                    if n0 < NN:
                        n1 = min(n0 + NPT, NN)
                        s, e = cstart[n0], cstart[n1]
                        m = e - s
                        assert m + 1 <= CAP
                        ob[1:1 + m] = np.searchsorted(dl, other[s:e])
                        nb[1:1 + m] = np.searchsorted(dl, own[s:e])
                        wrow[1:1 + m] = wp[s:e]
                        brow[1:1 + m] = bp[s:e]
                        arow[1:1 + m] = ap_[s:e]
                        cnts[:n1 - n0] = cnt[n0:n1]
                    ends = np.zeros(32, np.int16)
                    ends[:NPT] = np.cumsum(cnts).astype(np.int16)
                    sl = slice(16 * k, 16 * k + 16)
                    idxs[t, sl, 0:IDXW] = _wrap16(ob)
                    idxs[t, sl, IDXW:2 * IDXW] = _wrap16(nb)
                    idxs[t, sl, 2 * IDXW:] = _wrap16(ends)
                    prm[t, sl, 0:CAP] = wrow
                    prm[t, sl, CAP:2 * CAP] = brow
                    prm[t, sl, 2 * CAP:] = arow
        per_nc.append(dict(
            ctab=ctab,
            idxs=idxs,
            prm=_bf16(prm).astype(ml_dtypes.bfloat16),
        ))
    return dict(CAP=CAP, CLEN=CLEN), per_nc


def _build_program(CAP, CLEN, repeat=1):
    import sys
    if "/opt/trn_rl_repo" not in sys.path:
        sys.path.insert(0, "/opt/trn_rl_repo")
    from contextlib import ExitStack
    from concourse import bass, bacc, mybir, tile

    f32 = mybir.dt.float32
    bf16 = mybir.dt.bfloat16
    i16 = mybir.dt.int16
    Alu = mybir.AluOpType
    IDXW = CAP // 16

    nc = bacc.Bacc("TRN2", target_bir_lowering=False, debug=False,
                   num_devices=NCS)
    ctab_d = nc.dram_tensor("ctab_in", [ROUNDS, 128, CLEN], f32,
                            kind="ExternalInput")
    idxs_d = nc.dram_tensor("idxs_in", [TPC, 128, 2 * IDXW + 2], i16,
                            kind="ExternalInput")
    prm_d = nc.dram_tensor("prm_in", [TPC, 128, 3 * CAP], bf16,
                           kind="ExternalInput")
    out_d = nc.dram_tensor("res_out", [128, TPC * NPT], f32,
                           kind="ExternalOutput")

    with tile.TileContext(nc) as tc, ExitStack() as ctx:
        ctab_p = ctx.enter_context(tc.tile_pool(name="ctab", bufs=1))
        gat_p = ctx.enter_context(tc.tile_pool(name="gat", bufs=2))
        in_p = ctx.enter_context(tc.tile_pool(name="inp", bufs=2))
        zz_p = ctx.enter_context(tc.tile_pool(name="zz", bufs=2))
        p_p = ctx.enter_context(tc.tile_pool(name="pp", bufs=2))
        e_p = ctx.enter_context(tc.tile_pool(name="ee", bufs=2))
        res_p = ctx.enter_context(tc.tile_pool(name="res", bufs=1))

        res = res_p.tile([128, TPC * NPT], f32, tag="res")
        for _rep in range(repeat):
         for r in range(ROUNDS):
            ctab = ctab_p.tile([128, CLEN], f32, tag="ctab")
            nc.sync.dma_start(ctab[:], ctab_d.ap()[r])
            for ti in range(TPR):
                t = r * TPR + ti
                idx = in_p.tile([128, 2 * IDXW + 2], i16, tag="idx")
                nc.sync.dma_start(idx[:], idxs_d.ap()[t])
                prm = in_p.tile([128, 3 * CAP], bf16, tag="prm")
                nc.sync.dma_start(prm[:], prm_d.ap()[t])

                go = gat_p.tile([128, CAP], f32, tag="go")
                gn = gat_p.tile([128, CAP], f32, tag="gn")
                nc.gpsimd.ap_gather(go[:], ctab[:], idx[:, 0:IDXW],
                                    128, CLEN, 1, CAP)
                nc.gpsimd.ap_gather(gn[:], ctab[:], idx[:, IDXW:2 * IDXW],
                                    128, CLEN, 1, CAP)

                z1 = zz_p.tile([128, CAP], bf16, tag="zz")
                nc.vector.tensor_tensor(z1[:], gn[:], go[:], Alu.subtract)
                z2 = zz_p.tile([128, CAP], bf16, tag="zz")
                nc.vector.tensor_tensor(z2[:], z1[:], prm[:, 0:CAP], Alu.mult)
                z3 = zz_p.tile([128, CAP], bf16, tag="zz")
                nc.vector.tensor_tensor(z3[:], z2[:], prm[:, CAP:2 * CAP],
                                        Alu.add)
                th = zz_p.tile([128, CAP], bf16, tag="zz")
                nc.scalar.activation(th[:], z3[:],
                                     mybir.ActivationFunctionType.Tanh)
                cc = zz_p.tile([128, CAP], bf16, tag="zz")
                nc.vector.tensor_tensor(cc[:], th[:], prm[:, 2 * CAP:],
                                        Alu.mult)
                P = p_p.tile([128, CAP], f32, tag="P")
                nc.vector.tensor_tensor_scan(P[:], cc[:], cc[:], 0.0,
                                             Alu.add, Alu.bypass)
                Eb = e_p.tile([128, 48], f32, tag="Eb")
                nc.vector.memset(Eb[:, 0:1], 0.0)
                nc.gpsimd.ap_gather(Eb[:, 1:33], P[:],
                                    idx[:, 2 * IDXW:2 * IDXW + 2],
                                    128, CAP, 1, 32)
                nc.vector.tensor_tensor(res[:, t * NPT:(t + 1) * NPT],
                                        Eb[:, 1:1 + NPT], Eb[:, 0:NPT],
                                        Alu.subtract)
        nc.sync.dma_start(out_d.ap()[:], res[:])
    nc.compile()
    return nc


def kernel(**inputs) -> np.ndarray:
    import sys
    if "/opt/trn_rl_repo" not in sys.path:
        sys.path.insert(0, "/opt/trn_rl_repo")
    from concourse.bass_utils import run_bass_kernel_spmd

    x = np.asarray(inputs["x"], np.float32)
    param = np.asarray(inputs["param"], np.float32)
    meta, per_nc = _preprocess(x, param, inputs["src_node"],
                               inputs["des_node"])
    key = (meta["CAP"], meta["CLEN"])
    if key not in _cache:
        _cache[key] = _build_program(*key)
    nc = _cache[key]

    in_maps = [{"ctab_in": d["ctab"], "idxs_in": d["idxs"],
                "prm_in": d["prm"]} for d in per_nc]
    results = run_bass_kernel_spmd(nc, in_maps, list(range(NCS))).results

    full = np.zeros((B, NCS * NPNC), np.float32)
    for i, om in enumerate(results):
        o = om["res_out"]
        for k in range(QC):
            full[:, i * NPNC + k * NPC:i * NPNC + (k + 1) * NPC] = \
                o[16 * k:16 * k + 16]
    return np.ascontiguousarray(full[:, 1:NN])
